# revision 31
# baseline (speedup 1.0000x reference)
"""nn_AttnDecoder: LSTM+attention decoder, 8-core Trainium kernel.

The [T*B,512]@[512,32000] output projection (86% of FLOPs) runs on device,
tensor-parallel over vocab across 8 cores (4000 cols each). The final output
is masked by `lengths` (rows t >= lengths[b] are zero), so only the valid
(t,b) rows are computed: they are gathered host-side into a compact
[NV,512] matrix (NV = sum(lengths) ~ 579 of 1024 rows). The tiny
sequential scan (T=64, B=16) runs host-side.

Device kernel per core: out[NV, 4000] bf16 = hid[NV,512] @ VpT[512,4000],
K=512 contracted in 4 chunks of 128. Weights stream per 512-col vocab
slice (k-interleaved DRAM packing -> 4KB DMA lines) so matmuls start
before the full weight matrix lands; m-tiles are processed in pairs per
vocab sweep to overlap the weight stream with compute. PSUM->SBUF copies
rotate across Vector/GpSimd/Scalar engines; each m-tile row is written
back with a single 8KB-line DMA. A few warm-up matmuls on a memset tile
burn through the PE p-state ramp during the DMA lead-in.
"""
import numpy as np

DIM, DICT, B, T, S = 512, 32000, 16, 64, 64
N_CORES = 8
VSH = DICT // N_CORES          # 4000 vocab cols per core
NK = 4                         # K chunks of 128

_CACHE = {}
last_result = None


def _build_nc(nv, n_warm=8):
    import concourse.bacc as bacc
    import concourse.tile as tile
    import concourse.mybir as mybir

    f32 = mybir.dt.float32
    bf16 = mybir.dt.bfloat16

    n_m = -(-nv // 128)
    nvp = 128 * n_m
    # vocab slices: 7x512 + 416
    w_sizes = [512] * (VSH // 512) + ([VSH % 512] if VSH % 512 else [])
    w_offs = np.cumsum([0] + w_sizes).tolist()
    nw = len(w_sizes)
    # weight stream blocks, in slices: [n0], [n1,n2], [n3,n4], ... tail
    s_blocks = [[0]] + [list(range(j, min(j + 2, nw))) for j in range(1, nw, 2)]
    b_cols = [sum(w_sizes[n] for n in b) for b in s_blocks]
    b_offs = np.cumsum([0] + b_cols).tolist()
    blk_of = {}
    for j, b in enumerate(s_blocks):
        off = 0
        for n in b:
            blk_of[n] = (j, off)
            off += w_sizes[n]

    nc = bacc.Bacc(None, target_bir_lowering=False)
    hidT = nc.dram_tensor("hidT", [128, NK * nvp], bf16, kind="ExternalInput")
    vpT = nc.dram_tensor("vpT", [128, NK * VSH], bf16, kind="ExternalInput")
    out = nc.dram_tensor("out", [nvp, VSH], bf16, kind="ExternalOutput")

    with tile.TileContext(nc) as tc:
        with (
            tc.tile_pool(name="w", bufs=1) as wpool,
            tc.tile_pool(name="vps", bufs=2) as vppool,
            tc.tile_pool(name="ps", bufs=8, space="PSUM") as pspool,
            tc.tile_pool(name="rb", bufs=1) as rbpool,
        ):
            # PE warm-up on a zeroed tile while weights stream in: burns
            # through the p-state clock ramp so real matmuls run at 2.4GHz
            warm = wpool.tile([128, 576], bf16, name="warm", tag="warm")
            nc.gpsimd.memset(warm[:], 0.0)
            wps = pspool.tile([128, 512], f32, name="ps", tag="ps")
            for _ in range(n_warm):
                nc.tensor.matmul(wps[:64, :512], warm[:, :64], warm[:, 64:576],
                                 start=True, stop=True)

            hid_sb = wpool.tile([128, NK * nvp], bf16, name="hid", tag="hid")
            nc.sync.dma_start(hid_sb[:], hidT[:, :])

            # Weight blocks stream through a bufs=2 pool: blocks 0,1 load
            # upfront; block j+2's DMA is emitted (and so dependency-ordered)
            # after block j's last matmul, which via pool recycling makes its
            # trigger wait for exactly those readers. Concurrent DMA rings
            # share line slots round-robin, so this stagger is what makes the
            # weight stream arrive progressively instead of all-at-the-end.
            vp_sb = {}

            def load_blk(j):
                t = vppool.tile([128, NK * b_cols[j]], bf16,
                                name=f"vp{j}", tag="vp")
                nc.sync.dma_start(t[:], vpT[:, NK * b_offs[j]:NK * b_offs[j + 1]])
                vp_sb[j] = t

            load_blk(0)
            if len(s_blocks) > 1:
                load_blk(1)

            def do_copy(eng, dst, src):
                if eng is nc.scalar:
                    eng.copy(dst, src)
                else:
                    eng.tensor_copy(dst, src)

            cp_engines = [nc.vector, nc.scalar]
            n_cp = 0
            ms = list(range(n_m))
            last_m = ms[-1]
            # output rows split in three chunks per m-tile, cut so the DMA
            # descriptor generation (~0.9us per 128-line instruction,
            # serialized in the DGE) pipelines through the kernel instead of
            # bunching after the last copies
            ca = w_offs[min(3, nw)]
            cb = w_offs[min(6, nw)]
            cuts = sorted({0, ca, cb, VSH})
            spans = list(zip(cuts[:-1], cuts[1:]))
            rbs = {
                (m, lo): rbpool.tile([128, hi - lo], bf16,
                                     name=f"rb{m}_{lo}", tag=f"rb{m}_{lo}")
                for m in ms for lo, hi in spans
            }

            def span_of(col):
                for lo, hi in spans:
                    if lo <= col < hi:
                        return lo, hi
                raise AssertionError(col)

            for n, w in enumerate(w_sizes):
                bj, boff = blk_of[n]
                bw = b_cols[bj]
                for m in ms:
                    ps = pspool.tile([128, 512], f32, name="ps", tag="ps")
                    for k in range(NK):
                        c0 = k * bw + boff
                        nc.tensor.matmul(
                            ps[:, :w],
                            hid_sb[:, k * nvp + m * 128:k * nvp + (m + 1) * 128],
                            vp_sb[bj][:, c0:c0 + w],
                            start=(k == 0),
                            stop=(k == NK - 1),
                        )
                    lo, hi = span_of(w_offs[n])
                    dst = rbs[(m, lo)][:, w_offs[n] - lo:w_offs[n + 1] - lo]
                    if m == last_m and n == nw - 1:
                        # final tile: split the copy across both engines
                        h = w // 2
                        do_copy(nc.vector, dst[:, :h], ps[:, :h])
                        do_copy(nc.scalar, dst[:, h:w], ps[:, h:w])
                    else:
                        eng = cp_engines[n_cp % 2]
                        n_cp += 1
                        do_copy(eng, dst, ps[:, :w])
                    if w_offs[n + 1] == hi:
                        nc.sync.dma_start(out[m * 128:(m + 1) * 128, lo:hi],
                                          rbs[(m, lo)][:])
                # emit the DMA for block bj+2 now that block bj is fully read
                if n == s_blocks[bj][-1] and bj + 2 < len(s_blocks):
                    load_blk(bj + 2)
    nc.finalize()
    return nc


def _sigmoid(x):
    return 1.0 / (1.0 + np.exp(-x))


def kernel(words, lengths, input_len, pre_h, cell0, emb, W_ih, W_hh, b_ih, b_hh,
           W_h, W_s, b_s, v_t, V, b_V, Vp, b_Vp):
    global last_result
    from concourse.bass_utils import run_bass_kernel_spmd
    import ml_dtypes

    f8 = np.float64
    pre_h64 = pre_h.astype(f8)
    x_seq = emb.astype(f8)[words].transpose(1, 0, 2)          # [T,B,D]
    hid0 = pre_h64[input_len - 1, np.arange(B)]               # [B,D]
    Wh_pre = pre_h64 @ W_h.astype(f8).T                       # [S,B,D]
    kmask = np.arange(S)[:, None] < input_len[None, :]        # [S,B]

    X_gates = x_seq @ W_ih.astype(f8).T + (b_ih + b_hh).astype(f8)
    W_hhT = W_hh.astype(f8).T
    W_sT = W_s.astype(f8).T
    VT = V.astype(f8).T
    v0 = v_t.astype(f8)[0]

    h, c = hid0, cell0.astype(f8)
    hid_outs = np.empty((T, B, DIM), f8)
    for t in range(T):
        g = X_gates[t] + h @ W_hhT
        gi, gf, gg, go = np.split(g, 4, axis=-1)
        c = _sigmoid(gf) * c + _sigmoid(gi) * np.tanh(gg)
        h = _sigmoid(go) * np.tanh(c)
        q = c @ W_sT + b_s.astype(f8)
        e = np.tanh(Wh_pre + q[None]) @ v0                    # [S,B]
        e = np.where(kmask, e, -1e9)
        e = e - e.max(axis=0, keepdims=True)
        a = np.exp(e)
        a = a / a.sum(axis=0, keepdims=True)
        ctx = np.einsum('sb,sbd->bd', a, pre_h64)
        hid_outs[t] = np.concatenate([ctx, c], axis=1) @ VT + b_V.astype(f8)

    # gather valid (t,b) rows: final output is zero where t >= lengths[b]
    tmask = np.arange(T)[:, None] < np.asarray(lengths)[None, :]   # [T,B]
    valid = np.flatnonzero(tmask.ravel())                          # tb order
    nv = int(valid.size)
    nvp = -(-nv // 128) * 128

    hid_valid = np.zeros((nvp, DIM), np.float32)
    hid_valid[:nv] = hid_outs.reshape(T * B, DIM)[valid]
    # hidT [128, NK*nvp]: hidT[p, k*nvp + r] = hid_valid[r, k*128+p]
    hidT = np.ascontiguousarray(
        hid_valid.reshape(nvp, NK, 128).transpose(2, 1, 0).reshape(128, NK * nvp)
    ).astype(ml_dtypes.bfloat16)

    # vpT per core: per 512-col vocab slice, k-interleaved:
    # vpT[p, NK*w_off[n] + k*w + j] = Vp[core_off + n*512 + j, k*128 + p]
    w_sizes = [512] * (VSH // 512) + ([VSH % 512] if VSH % 512 else [])
    nw = len(w_sizes)
    s_blocks = [[0]] + [list(range(j, min(j + 2, nw))) for j in range(1, nw, 2)]
    b_cols = [sum(w_sizes[n] for n in b) for b in s_blocks]
    vp_bf = Vp.astype(ml_dtypes.bfloat16)
    in_maps = []
    for i in range(N_CORES):
        vc = vp_bf[i * VSH:(i + 1) * VSH]                     # [VSH, 512]
        blocks, off = [], 0
        for w in b_cols:
            blk = vc[off:off + w].reshape(w, NK, 128)         # [w, k, p]
            blocks.append(blk.transpose(2, 1, 0).reshape(128, NK * w))
            off += w
        vpc = np.ascontiguousarray(np.concatenate(blocks, axis=1))
        in_maps.append({"hidT": hidT, "vpT": vpc})

    key = ("nc", nv)
    if key not in _CACHE:
        _CACHE[key] = _build_nc(nv)
    res = run_bass_kernel_spmd(_CACHE[key], in_maps, core_ids=list(range(N_CORES)))
    last_result = res

    valid_out = np.empty((nv, DICT), np.float32)
    for i in range(N_CORES):
        valid_out[:, i * VSH:(i + 1) * VSH] = res.results[i]["out"][:nv]
    valid_out += b_Vp.astype(np.float32)[None, :]
    full = np.zeros((T * B, DICT), np.float32)
    full[valid] = valid_out
    return full.reshape(T, B, DICT)


# revision 33
# speedup vs baseline: 1.0993x; 1.0993x over previous
"""nn_AttnDecoder: LSTM+attention decoder, 8-core Trainium kernel.

The [T*B,512]@[512,32000] output projection (86% of FLOPs) runs on device,
tensor-parallel over vocab across 8 cores (4000 cols each). The final output
is masked by `lengths` (rows t >= lengths[b] are zero), so only the valid
(t,b) rows are computed: they are gathered host-side into a compact
[NV,512] matrix (NV = sum(lengths) ~ 579 of 1024 rows). The tiny
sequential scan (T=64, B=16) runs host-side.

Device kernel per core: out[NV, 4000] bf16 = hid[NV,512] @ VpT[512,4000],
K=512 contracted in 4 chunks of 128. Weights stream per 512-col vocab
slice (k-interleaved DRAM packing -> 4KB DMA lines) so matmuls start
before the full weight matrix lands; m-tiles are processed in pairs per
vocab sweep to overlap the weight stream with compute. PSUM->SBUF copies
rotate across Vector/GpSimd/Scalar engines; each m-tile row is written
back with a single 8KB-line DMA. A few warm-up matmuls on a memset tile
burn through the PE p-state ramp during the DMA lead-in.
"""
import numpy as np

DIM, DICT, B, T, S = 512, 32000, 16, 64, 64
N_CORES = 8
VSH = DICT // N_CORES          # 4000 vocab cols per core
NK = 4                         # K chunks of 128

_CACHE = {}
last_result = None


def _build_nc(nv, n_warm=8):
    import concourse.bacc as bacc
    import concourse.tile as tile
    import concourse.mybir as mybir

    f32 = mybir.dt.float32
    bf16 = mybir.dt.bfloat16

    n_m = -(-nv // 128)
    nvp = 128 * n_m
    # vocab slices: 7x512 + 416
    w_sizes = [512] * (VSH // 512) + ([VSH % 512] if VSH % 512 else [])
    w_offs = np.cumsum([0] + w_sizes).tolist()
    nw = len(w_sizes)
    # weight stream blocks, in slices: [n0], [n1,n2], [n3,n4], ... tail
    s_blocks = [[0]] + [list(range(j, min(j + 2, nw))) for j in range(1, nw, 2)]
    b_cols = [sum(w_sizes[n] for n in b) for b in s_blocks]
    b_offs = np.cumsum([0] + b_cols).tolist()
    blk_of = {}
    for j, b in enumerate(s_blocks):
        off = 0
        for n in b:
            blk_of[n] = (j, off)
            off += w_sizes[n]

    nc = bacc.Bacc(None, target_bir_lowering=False)
    hidT = nc.dram_tensor("hidT", [128, NK * nvp], bf16, kind="ExternalInput")
    vpT = nc.dram_tensor("vpT", [128, NK * VSH], bf16, kind="ExternalInput")
    out = nc.dram_tensor("out", [nvp, VSH], bf16, kind="ExternalOutput")

    with tile.TileContext(nc) as tc:
        with (
            tc.tile_pool(name="w", bufs=1) as wpool,
            tc.tile_pool(name="vps", bufs=2) as vppool,
            tc.tile_pool(name="ps", bufs=8, space="PSUM") as pspool,
            tc.tile_pool(name="rb", bufs=1) as rbpool,
        ):
            # PE warm-up on a zeroed tile while weights stream in: burns
            # through the p-state clock ramp so real matmuls run at 2.4GHz
            warm = wpool.tile([128, 576], bf16, name="warm", tag="warm")
            nc.gpsimd.memset(warm[:], 0.0)
            wps = pspool.tile([128, 512], f32, name="ps", tag="ps")
            for _ in range(n_warm):
                nc.tensor.matmul(wps[:64, :512], warm[:, :64], warm[:, 64:576],
                                 start=True, stop=True)

            # hid loads via the Activation engine's DGE so its descriptor
            # generation runs in parallel with the sync-DGE weight blocks
            hid_sb = wpool.tile([128, NK * nvp], bf16, name="hid", tag="hid")
            nc.scalar.dma_start(hid_sb[:], hidT[:, :])

            # Weight blocks stream through a bufs=2 pool: blocks 0,1 load
            # upfront; block j+2's DMA is emitted (and so dependency-ordered)
            # after block j's last matmul, which via pool recycling makes its
            # trigger wait for exactly those readers. Concurrent DMA rings
            # share line slots round-robin, so this stagger is what makes the
            # weight stream arrive progressively instead of all-at-the-end.
            vp_sb = {}

            def load_blk(j):
                t = vppool.tile([128, NK * b_cols[j]], bf16,
                                name=f"vp{j}", tag="vp")
                nc.sync.dma_start(t[:], vpT[:, NK * b_offs[j]:NK * b_offs[j + 1]])
                vp_sb[j] = t

            load_blk(0)
            if len(s_blocks) > 1:
                load_blk(1)

            def do_copy(eng, dst, src):
                if eng is nc.scalar:
                    eng.copy(dst, src)
                else:
                    eng.tensor_copy(dst, src)

            cp_engines = [nc.vector, nc.scalar]
            n_cp = 0
            ms = list(range(n_m))
            last_m = ms[-1]
            # output rows split in three chunks per m-tile, cut so the DMA
            # descriptor generation (~0.9us per 128-line instruction,
            # serialized in the DGE) pipelines through the kernel instead of
            # bunching after the last copies
            ca = w_offs[min(3, nw)]
            cb = w_offs[min(6, nw)]
            cuts = sorted({0, ca, cb, VSH})
            spans = list(zip(cuts[:-1], cuts[1:]))
            rbs = {
                (m, lo): rbpool.tile([128, hi - lo], bf16,
                                     name=f"rb{m}_{lo}", tag=f"rb{m}_{lo}")
                for m in ms for lo, hi in spans
            }

            def span_of(col):
                for lo, hi in spans:
                    if lo <= col < hi:
                        return lo, hi
                raise AssertionError(col)

            for n, w in enumerate(w_sizes):
                bj, boff = blk_of[n]
                bw = b_cols[bj]
                for m in ms:
                    ps = pspool.tile([128, 512], f32, name="ps", tag="ps")
                    for k in range(NK):
                        c0 = k * bw + boff
                        nc.tensor.matmul(
                            ps[:, :w],
                            hid_sb[:, k * nvp + m * 128:k * nvp + (m + 1) * 128],
                            vp_sb[bj][:, c0:c0 + w],
                            start=(k == 0),
                            stop=(k == NK - 1),
                        )
                    lo, hi = span_of(w_offs[n])
                    dst = rbs[(m, lo)][:, w_offs[n] - lo:w_offs[n + 1] - lo]
                    if m == last_m and n == nw - 1:
                        # final tile: split the copy across both engines
                        h = w // 2
                        do_copy(nc.vector, dst[:, :h], ps[:, :h])
                        do_copy(nc.scalar, dst[:, h:w], ps[:, h:w])
                    else:
                        eng = cp_engines[n_cp % 2]
                        n_cp += 1
                        do_copy(eng, dst, ps[:, :w])
                    if w_offs[n + 1] == hi:
                        # alternate out-wave triggers across the two DGEs so
                        # their descriptor generation runs in parallel
                        wave = spans.index((lo, hi))
                        deng = nc.scalar if (wave + m) % 2 else nc.sync
                        deng.dma_start(out[m * 128:(m + 1) * 128, lo:hi],
                                       rbs[(m, lo)][:])
                # emit the DMA for block bj+2 now that block bj is fully read
                if n == s_blocks[bj][-1] and bj + 2 < len(s_blocks):
                    load_blk(bj + 2)
    nc.finalize()
    return nc


def _sigmoid(x):
    return 1.0 / (1.0 + np.exp(-x))


def kernel(words, lengths, input_len, pre_h, cell0, emb, W_ih, W_hh, b_ih, b_hh,
           W_h, W_s, b_s, v_t, V, b_V, Vp, b_Vp):
    global last_result
    from concourse.bass_utils import run_bass_kernel_spmd
    import ml_dtypes

    f8 = np.float64
    pre_h64 = pre_h.astype(f8)
    x_seq = emb.astype(f8)[words].transpose(1, 0, 2)          # [T,B,D]
    hid0 = pre_h64[input_len - 1, np.arange(B)]               # [B,D]
    Wh_pre = pre_h64 @ W_h.astype(f8).T                       # [S,B,D]
    kmask = np.arange(S)[:, None] < input_len[None, :]        # [S,B]

    X_gates = x_seq @ W_ih.astype(f8).T + (b_ih + b_hh).astype(f8)
    W_hhT = W_hh.astype(f8).T
    W_sT = W_s.astype(f8).T
    VT = V.astype(f8).T
    v0 = v_t.astype(f8)[0]

    h, c = hid0, cell0.astype(f8)
    hid_outs = np.empty((T, B, DIM), f8)
    for t in range(T):
        g = X_gates[t] + h @ W_hhT
        gi, gf, gg, go = np.split(g, 4, axis=-1)
        c = _sigmoid(gf) * c + _sigmoid(gi) * np.tanh(gg)
        h = _sigmoid(go) * np.tanh(c)
        q = c @ W_sT + b_s.astype(f8)
        e = np.tanh(Wh_pre + q[None]) @ v0                    # [S,B]
        e = np.where(kmask, e, -1e9)
        e = e - e.max(axis=0, keepdims=True)
        a = np.exp(e)
        a = a / a.sum(axis=0, keepdims=True)
        ctx = np.einsum('sb,sbd->bd', a, pre_h64)
        hid_outs[t] = np.concatenate([ctx, c], axis=1) @ VT + b_V.astype(f8)

    # gather valid (t,b) rows: final output is zero where t >= lengths[b]
    tmask = np.arange(T)[:, None] < np.asarray(lengths)[None, :]   # [T,B]
    valid = np.flatnonzero(tmask.ravel())                          # tb order
    nv = int(valid.size)
    nvp = -(-nv // 128) * 128

    hid_valid = np.zeros((nvp, DIM), np.float32)
    hid_valid[:nv] = hid_outs.reshape(T * B, DIM)[valid]
    # hidT [128, NK*nvp]: hidT[p, k*nvp + r] = hid_valid[r, k*128+p]
    hidT = np.ascontiguousarray(
        hid_valid.reshape(nvp, NK, 128).transpose(2, 1, 0).reshape(128, NK * nvp)
    ).astype(ml_dtypes.bfloat16)

    # vpT per core: per 512-col vocab slice, k-interleaved:
    # vpT[p, NK*w_off[n] + k*w + j] = Vp[core_off + n*512 + j, k*128 + p]
    w_sizes = [512] * (VSH // 512) + ([VSH % 512] if VSH % 512 else [])
    nw = len(w_sizes)
    s_blocks = [[0]] + [list(range(j, min(j + 2, nw))) for j in range(1, nw, 2)]
    b_cols = [sum(w_sizes[n] for n in b) for b in s_blocks]
    vp_bf = Vp.astype(ml_dtypes.bfloat16)
    in_maps = []
    for i in range(N_CORES):
        vc = vp_bf[i * VSH:(i + 1) * VSH]                     # [VSH, 512]
        blocks, off = [], 0
        for w in b_cols:
            blk = vc[off:off + w].reshape(w, NK, 128)         # [w, k, p]
            blocks.append(blk.transpose(2, 1, 0).reshape(128, NK * w))
            off += w
        vpc = np.ascontiguousarray(np.concatenate(blocks, axis=1))
        in_maps.append({"hidT": hidT, "vpT": vpc})

    key = ("nc", nv)
    if key not in _CACHE:
        _CACHE[key] = _build_nc(nv)
    res = run_bass_kernel_spmd(_CACHE[key], in_maps, core_ids=list(range(N_CORES)))
    last_result = res

    valid_out = np.empty((nv, DICT), np.float32)
    for i in range(N_CORES):
        valid_out[:, i * VSH:(i + 1) * VSH] = res.results[i]["out"][:nv]
    valid_out += b_Vp.astype(np.float32)[None, :]
    full = np.zeros((T * B, DICT), np.float32)
    full[valid] = valid_out
    return full.reshape(T, B, DICT)
